# revision 1
# baseline (speedup 1.0000x reference)
"""CrossAttention Trainium2 kernel.

Full inputs -> full output. Sharding: 8 cores = 4 batches x 2 head-groups
(8 heads each). Per core:

  Phase A (PE-heavy): PE-transpose x/context into cin-major layout, then
  project kT[dout,keys], qT[dout,qrows] (SCALE folded into Wq on host)
  and v_ext[keys, (v|1) per head] with bf16 matmuls.

  Phase B (ACT-paced): per head-pair/q-chunk/key-tile
    scoresT[key, qrow] = kT.T @ qT   (two heads on disjoint PE row groups)
    attnT = exp(scoresT)             (no max-subtraction: |scores| <~ 3)
    O^T accumulates (v|1).T @ attnT  -> row 64 = softmax denominator
    out = O^T[0:64] * (1/O^T[64])    broadcast via K=1 matmul

  Only head-pair 0's kT/qT strips are computed up front; the remaining
  projection matmuls are injected as PE filler inside phase B's
  ACT-paced loop so both engines stay busy.
"""

import numpy as np

B, NQ, NC = 4, 2048, 2048
QDIM = CDIM = 1024
H, D = 16, 64
SCALE = D**-0.5
P = 128
HG = 8            # heads per core
DG = HG * D       # 512 output dims per core
N_CORES = 8

MM_DTYPE = "bf16"   # "bf16" | "f32r"

_PROGRAM = None


def _build_program(reps_a=None, reps_b=None, mm_dtype=None):
    import contextlib
    import concourse.mybir as mybir
    import concourse.tile as tile
    from concourse import bacc
    from concourse.masks import make_identity

    mm = mm_dtype or MM_DTYPE
    f32 = mybir.dt.float32
    f32r = mybir.dt.float32r
    bf16 = mybir.dt.bfloat16
    mdt = bf16 if mm == "bf16" else f32r
    AF = mybir.ActivationFunctionType

    nc = bacc.Bacc("TRN2", target_bir_lowering=False, debug=False,
                   num_devices=N_CORES)

    wdt = f32 if mm == "bf16" else f32r
    x_nat = nc.dram_tensor("x_nat", [NQ, QDIM], f32, kind="ExternalInput")
    ctx_nat = nc.dram_tensor("ctx_nat", [NC, CDIM], f32, kind="ExternalInput")
    wq = nc.dram_tensor("wq", [QDIM, DG], wdt, kind="ExternalInput")
    wk = nc.dram_tensor("wk", [CDIM, DG], wdt, kind="ExternalInput")
    wv = nc.dram_tensor("wv", [CDIM, DG], wdt, kind="ExternalInput")
    bq2 = nc.dram_tensor("bq2", [P, 4], f32, kind="ExternalInput")
    bk2 = nc.dram_tensor("bk2", [P, 4], f32, kind="ExternalInput")
    bvb = nc.dram_tensor("bvb", [P, DG], f32, kind="ExternalInput")
    out_T = nc.dram_tensor("out_T", [DG, NQ], f32, kind="ExternalOutput")

    def wdma(out_ap, dram_ap):
        # bf16 weight load casts f32->bf16, which needs SWDGE (gpsimd)
        if mm == "bf16":
            nc.gpsimd.dma_start(out_ap, dram_ap)
        else:
            nc.sync.dma_start(out_ap, dram_ap)

    with tile.TileContext(nc) as tc:
        with (
            tc.tile_pool(name="const", bufs=1) as const_pool,
            tc.tile_pool(name="persist", bufs=1) as persist,
            tc.tile_pool(name="tpool", bufs=2) as t_pool,
            tc.tile_pool(name="wpool", bufs=1) as w_pool,
            tc.tile_pool(name="stream", bufs=3) as stream_pool,
            tc.tile_pool(name="att", bufs=4) as att_pool,
            tc.tile_pool(name="outp", bufs=2) as out_pool,
            tc.tile_pool(name="small", bufs=2) as small_pool,
            tc.tile_pool(name="ps_acc", bufs=3, space="PSUM") as ps_acc,
            tc.tile_pool(name="ps_o", bufs=1, space="PSUM") as ps_o,
        ):
            ident = const_pool.tile([P, P], f32)
            make_identity(nc, ident)
            ones_f32 = const_pool.tile([1, 64], f32)
            nc.vector.memset(ones_f32[:], 1.0)
            ones_col = const_pool.tile([1, 64], f32r)
            nc.vector.tensor_copy(ones_col[:], ones_f32[:])
            bq_sb = const_pool.tile([P, 4], f32)
            bk_sb = const_pool.tile([P, 4], f32)
            bvb_sb = const_pool.tile([P, DG], f32)
            nc.sync.dma_start(bq_sb[:], bq2[:])
            nc.sync.dma_start(bk_sb[:], bk2[:])
            nc.sync.dma_start(bvb_sb[:], bvb[:])

            # persistent activations; strip t = douts [128t, 128t+128)
            # = head pair (2t, 2t+1).  Separate tiles per strip so Tile's
            # dependency tracking stays per-strip.
            kTs = [persist.tile([P, NC], mdt, name=f"kT{t}")
                   for t in range(4)]
            qTs = [persist.tile([P, NQ], mdt, name=f"qT{t}")
                   for t in range(4)]
            # v strip per keytile: head h at cols [65h, 65h+64), ones
            # column at 65h+64.  One tile per keytile keeps dependency
            # tracking per-strip so late V strips can ride the filler queue.
            v_exts = [persist.tile([P, HG * 65], mdt, name=f"v_ext{kt}")
                      for kt in range(16)]
            ones_src = const_pool.tile([P, HG], f32)
            nc.vector.memset(ones_src[:], 1.0)
            for kt in range(16):
                nc.vector.tensor_copy(
                    v_exts[kt][:].rearrange("p (h c) -> p h c", c=65)
                    [:, :, 64],
                    ones_src[:])

            # weights all resident (bf16 halves the footprint)
            wk_sb = w_pool.tile([P, 8, DG], mdt, tag="wk")
            wv_sb = w_pool.tile([P, 8, DG], mdt, tag="wv")
            wq_sb = w_pool.tile([P, 8, DG], mdt, tag="wq")

            def loop_a():
                if reps_a is None:
                    return contextlib.nullcontext()
                return tc.For_i(0, reps_a, 1)

            def loop_b():
                if reps_b is None:
                    return contextlib.nullcontext()
                return tc.For_i(0, reps_b, 1)

            def load_transpose_half(src_dram, half, tag):
                # returns [cin partition, cin-strip, 1024] = block-transpose
                # of src[half*1024:(half+1)*1024, :].  Loads ride HWDGE in
                # f32 (keeps the SWDGE queue free for the weight casts);
                # the PSUM->SBUF copy does the f32 -> mdt rounding.
                tTh = t_pool.tile([P, 8, 1024], mdt, tag=tag,
                                  name=f"{tag}_{half}")
                for t8 in range(8):
                    row0 = (half * 8 + t8) * P
                    ct = stream_pool.tile([P, CDIM], f32, tag="ct")
                    nc.sync.dma_start(ct[:], src_dram[row0:row0 + P, :])
                    for g in range(2):
                        pt = ps_acc.tile([P, 512], f32, tag="pacc")
                        for i in range(4):
                            c = g * 4 + i
                            nc.tensor.transpose(
                                pt[:, i * P:(i + 1) * P],
                                ct[:, c * P:(c + 1) * P], ident[:])
                        nc.vector.tensor_copy(
                            tTh[:, g * 4:(g + 1) * 4, t8 * P:(t8 + 1) * P],
                            pt[:].rearrange("p (a b) -> p a b", b=P))
                return tTh

            def emit_kq_chunk(dst, w_sb, b_sb, tT, t, half, kc2):
                # one [128, 512] chunk of kT/qT strip t
                pk = ps_acc.tile([P, 512], f32, tag="pacc",
                                 name=f"pk_{dst.name}_{half}_{kc2}")
                for c in range(8):
                    nc.tensor.matmul(
                        pk[:],
                        w_sb[:, c, t * P:(t + 1) * P],
                        tT[:, c, kc2 * 512:(kc2 + 1) * 512],
                        start=(c == 0), stop=(c == 7))
                col0 = half * 1024 + kc2 * 512
                nc.vector.tensor_scalar_add(
                    dst[:, col0:col0 + 512], pk[:], b_sb[:, t:t + 1])

            def emit_v_strip(tT, half, t8):
                kt = half * 8 + t8
                pv = ps_acc.tile([P, 512], f32, tag="pacc",
                                 name=f"pv_{kt}")
                for c in range(8):
                    nc.tensor.matmul(
                        pv[:],
                        tT[:, c, t8 * P:(t8 + 1) * P],
                        wv_sb[:, c, :],
                        start=(c == 0), stop=(c == 7))
                for h in range(HG):
                    nc.vector.tensor_add(
                        v_exts[kt][:, h * 65:h * 65 + 64],
                        pv[:, h * 64:(h + 1) * 64],
                        bvb_sb[:, h * 64:(h + 1) * 64])

            # ---------------- Phase A: upfront work ----------------
            with loop_a():
                for c in range(8):
                    wdma(wv_sb[:, c, :], wv[c * P:(c + 1) * P, :])
                    wdma(wk_sb[:, c, :], wk[c * P:(c + 1) * P, :])
                    wdma(wq_sb[:, c, :], wq[c * P:(c + 1) * P, :])
                tTc = [load_transpose_half(ctx_nat, h, "tTc")
                       for h in range(2)]
                # kT strip 0
                for half in range(2):
                    for kc2 in range(2):
                        emit_kq_chunk(kTs[0], wk_sb, bk_sb, tTc[half],
                                      0, half, kc2)
                # V strips 0-7 (strips 8-15 ride the filler queue; B's
                # qc0 o-pair for keytile kt runs around iteration kt+1)
                for t8 in range(8):
                    emit_v_strip(tTc[0], 0, t8)
                tTx = [load_transpose_half(x_nat, h, "tTx")
                       for h in range(2)]
                # qT strip 0
                for half in range(2):
                    for qc2 in range(2):
                        emit_kq_chunk(qTs[0], wq_sb, bq_sb, tTx[half],
                                      0, half, qc2)

            # remaining work, injected as PE filler in phase B.
            # V strips 8-15 first: B's qc0 o-pair for keytile kt runs at
            # iteration ~kt+1, and fillers drain one per 2 iterations
            # starting at iteration 2, so strip 8+i lands at iteration
            # 2i+2 <= its first use at iteration ~9+i.
            filler = [(emit_v_strip, (tTc[1], 1, t8)) for t8 in range(8)]
            for t in (1, 2, 3):
                for half in range(2):
                    for kc2 in range(2):
                        filler.append((emit_kq_chunk,
                                       (kTs[t], wk_sb, bk_sb, tTc[half],
                                        t, half, kc2)))
                for half in range(2):
                    for qc2 in range(2):
                        filler.append((emit_kq_chunk,
                                       (qTs[t], wq_sb, bq_sb, tTx[half],
                                        t, half, qc2)))

            # ---------------- Phase B: attention ----------------
            with loop_b():
                fill_idx = [0]

                def maybe_fill():
                    if fill_idx[0] < len(filler):
                        fn, args = filler[fill_idx[0]]
                        fn(*args)
                        fill_idx[0] += 1

                it = [0]
                pending_norm = [None]

                def flush_norm():
                    if pending_norm[0] is not None:
                        pending_norm[0]()
                        pending_norm[0] = None

                for hp in range(4):
                    o_sb = [out_pool.tile([64, NQ], f32, tag=f"o{j}",
                                          name=f"o_sb{hp}_{j}")
                            for j in range(2)]
                    for qc in range(4):
                        po = [ps_o.tile([65, 512], f32, tag=f"po{j}",
                                        name=f"po{hp}_{qc}_{j}")
                              for j in range(2)]

                        def emit_opair(at_prev, kt_prev, po=po, hp=hp):
                            for j in range(2):
                                nc.tensor.matmul(
                                    po[j][:],
                                    v_exts[kt_prev][
                                        :, (2 * hp + j) * 65:
                                        (2 * hp + j) * 65 + 65],
                                    at_prev[:, j * 512:(j + 1) * 512],
                                    start=(kt_prev == 0),
                                    stop=(kt_prev == 15))

                        prev = None
                        for kt in range(16):
                            ps_pair = ps_acc.tile([P, 1024], f32, tag="pacc",
                                                  name=f"ps{hp}_{qc}_{kt}")
                            for j in range(2):
                                nc.tensor.matmul(
                                    ps_pair[:, j * 512:(j + 1) * 512],
                                    kTs[hp][j * 64:(j + 1) * 64,
                                            kt * P:(kt + 1) * P],
                                    qTs[hp][j * 64:(j + 1) * 64,
                                            qc * 512:(qc + 1) * 512],
                                    start=True, stop=True,
                                    tile_position=(j * 64, 0))
                            if kt == 1:
                                # normalize the previous q-chunk now; its
                                # PE op queues behind this chunk's scores
                                flush_norm()
                            elif it[0] % (2 if fill_idx[0] < 8 else 4) == 0:
                                maybe_fill()
                            it[0] += 1
                            if prev is not None:
                                emit_opair(*prev)
                            at = att_pool.tile([P, 1024], mdt, tag="at",
                                               name=f"at{hp}_{qc}_{kt}")
                            nc.scalar.activation(at[:], ps_pair[:], AF.Exp)
                            prev = (at, kt)
                        emit_opair(*prev)

                        def norm(po=po, o_sb=o_sb, hp=hp, qc=qc):
                            for j in range(2):
                                rec = small_pool.tile(
                                    [1, 512], f32r, tag=f"rec{j}",
                                    name=f"rec{hp}_{qc}_{j}")
                                with nc.allow_low_precision(
                                        reason="f32r recip for bcast mm"):
                                    nc.vector.reciprocal(rec[:],
                                                         po[j][64:65, :])
                                pr = ps_acc.tile([64, 512], f32, tag="pacc",
                                                 name=f"pr{hp}_{qc}_{j}")
                                nc.tensor.matmul(pr[:], ones_col[:], rec[:],
                                                 start=True, stop=True)
                                rb = small_pool.tile([64, 512], f32,
                                                     tag=f"rb{j}",
                                                     name=f"rb{hp}_{qc}_{j}")
                                nc.vector.tensor_copy(rb[:], pr[:])
                                nc.vector.tensor_mul(
                                    o_sb[j][:, qc * 512:(qc + 1) * 512],
                                    po[j][0:64, :], rb[:])
                        pending_norm[0] = norm
                    flush_norm()
                    for j in range(2):
                        h0 = (2 * hp + j) * 64
                        nc.sync.dma_start(out_T[h0:h0 + 64, :], o_sb[j][:])

    nc.compile()
    return nc


def _get_program():
    global _PROGRAM
    if _PROGRAM is None:
        _PROGRAM = _build_program()
    return _PROGRAM


def _numpy_fallback(x, context, mask, Wq, bq, Wk, bk, Wv, bv):
    out = np.empty((B, NQ, H * D), np.float32)
    for b in range(B):
        q = (x[b] @ Wq + bq).reshape(NQ, H, D)
        k = (context[b] @ Wk + bk).reshape(NC, H, D)
        v = (context[b] @ Wv + bv).reshape(NC, H, D)
        m = mask[b].astype(bool)
        for h in range(H):
            s = (q[:, h] @ k[:, h].T) * SCALE
            s = np.where(m[None, :], s, -np.finfo(np.float32).max)
            s = s - s.max(-1, keepdims=True)
            e = np.exp(s)
            a = e / e.sum(-1, keepdims=True)
            out[b, :, h * D:(h + 1) * D] = a @ v[:, h]
    return out


def make_in_maps(x, context, Wq, bq, Wk, bk, Wv, bv):
    in_maps = []
    for c in range(N_CORES):
        b, hg = divmod(c, 2)
        sl = slice(hg * DG, (hg + 1) * DG)
        in_maps.append({
            "x_nat": np.ascontiguousarray(x[b], np.float32),
            "ctx_nat": np.ascontiguousarray(context[b], np.float32),
            "wq": np.ascontiguousarray(Wq[:, sl] * SCALE, np.float32),
            "wk": np.ascontiguousarray(Wk[:, sl], np.float32),
            "wv": np.ascontiguousarray(Wv[:, sl], np.float32),
            # strip t of kT/qT gets bias for douts [128t, 128t+128)
            "bq2": np.ascontiguousarray(
                (bq[sl] * SCALE).reshape(4, P).T, np.float32),
            "bk2": np.ascontiguousarray(bk[sl].reshape(4, P).T, np.float32),
            "bvb": np.ascontiguousarray(
                np.broadcast_to(bv[sl], (P, DG)), np.float32),
        })
    return in_maps


def assemble_output(results):
    out = np.empty((B, NQ, H * D), np.float32)
    for c in range(N_CORES):
        b, hg = divmod(c, 2)
        out[b, :, hg * DG:(hg + 1) * DG] = results[c]["out_T"].T
    return out


def kernel(x, context, mask, Wq, bq, Wk, bk, Wv, bv):
    x = np.asarray(x, np.float32)
    context = np.asarray(context, np.float32)
    mask = np.asarray(mask)
    Wq = np.asarray(Wq, np.float32)
    bq = np.asarray(bq, np.float32)
    Wk = np.asarray(Wk, np.float32)
    bk = np.asarray(bk, np.float32)
    Wv = np.asarray(Wv, np.float32)
    bv = np.asarray(bv, np.float32)

    if not mask.all():
        return _numpy_fallback(x, context, mask, Wq, bq, Wk, bk, Wv, bv)

    from concourse.bass_utils import run_bass_kernel_spmd

    nc = _get_program()
    in_maps = make_in_maps(x, context, Wq, bq, Wk, bk, Wv, bv)
    res = run_bass_kernel_spmd(nc, in_maps, core_ids=list(range(N_CORES)))
    return assemble_output(res.results)



# revision 4
# speedup vs baseline: 1.0558x; 1.0558x over previous
"""CrossAttention Trainium2 kernel.

Full inputs -> full output. Sharding: 8 cores = 4 batches x 2 head-groups
(8 heads each). Per core:

  Inputs x/context arrive pre-cast to bf16 (host) and are loaded with the
  X-bar DMA-transpose directly into cin-major layout (no PE transposes).
  Weights arrive pre-cast bf16 (wq pre-scaled by D**-0.5).

  Phase A: DMA-transpose x/ctx (4 quarter-tensor calls each, HWDGE ring),
  weights on the SWDGE ring, then PE projects kT strip 0, V strips 0-12
  and qT strip 0, each chunk gated only on its input quarter.

  Phase B (ACT-paced): per head-pair/q-chunk/key-tile
    scoresT[key, qrow] = kT.T @ qT   (two heads concurrent on disjoint PE
                                      row groups via tile_position)
    attnT = exp(scoresT)             (no max-subtraction: |scores| <~ 3)
    O^T += v_ext.T @ attnT           v_ext padded to 128 weight columns
                                     (FWL) with a ones column at col 64
                                     -> po row 64 = softmax denominator
    out = po[0:64] * (1/po[64])      broadcast via K=1 matmul

  Remaining projections (V strips 13-15, kT/qT strips 1-3) are split into
  ~450ns granules (8 matmuls of n=128) and injected one per iteration as
  PE filler inside the ACT-paced loop.
"""

import numpy as np

B, NQ, NC = 4, 2048, 2048
QDIM = CDIM = 1024
H, D = 16, 64
SCALE = D**-0.5
P = 128
HG = 8            # heads per core
DG = HG * D       # 512 output dims per core
N_CORES = 8

_PROGRAM = None


def _build_program(reps_a=None, reps_b=None):
    import contextlib
    import concourse.mybir as mybir
    import concourse.tile as tile
    from concourse import bacc

    f32 = mybir.dt.float32
    f32r = mybir.dt.float32r
    bf16 = mybir.dt.bfloat16
    AF = mybir.ActivationFunctionType

    nc = bacc.Bacc("TRN2", target_bir_lowering=False, debug=False,
                   num_devices=N_CORES)

    x_bf = nc.dram_tensor("x_bf", [NQ, QDIM], bf16, kind="ExternalInput")
    ctx_bf = nc.dram_tensor("ctx_bf", [NC, CDIM], bf16, kind="ExternalInput")
    wq = nc.dram_tensor("wq", [QDIM, DG], bf16, kind="ExternalInput")
    wk = nc.dram_tensor("wk", [CDIM, DG], bf16, kind="ExternalInput")
    wv = nc.dram_tensor("wv", [CDIM, DG], bf16, kind="ExternalInput")
    bq2 = nc.dram_tensor("bq2", [P, 4], f32, kind="ExternalInput")
    bk2 = nc.dram_tensor("bk2", [P, 4], f32, kind="ExternalInput")
    bvb = nc.dram_tensor("bvb", [P, DG], f32, kind="ExternalInput")
    out_T = nc.dram_tensor("out_T", [DG, NQ], f32, kind="ExternalOutput")

    with tile.TileContext(nc) as tc:
        with (
            tc.tile_pool(name="const", bufs=1) as const_pool,
            tc.tile_pool(name="persist", bufs=1) as persist,
            tc.tile_pool(name="wpool", bufs=1) as w_pool,
            tc.tile_pool(name="att", bufs=4) as att_pool,
            tc.tile_pool(name="outp", bufs=2) as out_pool,
            tc.tile_pool(name="small", bufs=2) as small_pool,
            tc.tile_pool(name="ps_acc", bufs=3, space="PSUM") as ps_acc,
            tc.tile_pool(name="ps_o", bufs=1, space="PSUM") as ps_o,
        ):
            ones_f32 = const_pool.tile([1, 64], f32)
            nc.vector.memset(ones_f32[:], 1.0)
            ones_col = const_pool.tile([1, 64], f32r)
            nc.vector.tensor_copy(ones_col[:], ones_f32[:])
            bq_sb = const_pool.tile([P, 4], f32)
            bk_sb = const_pool.tile([P, 4], f32)
            bvb_sb = const_pool.tile([P, DG], f32)
            ones_src = const_pool.tile([P, HG], f32)
            nc.vector.memset(ones_src[:], 1.0)

            # transposed inputs, one tile per 512-row quarter:
            # tT*_q[q][cin%128, cin//128, row-512q] = src[row, cin]
            tTc_q = [persist.tile([P, 8, 512], bf16, name=f"tTc{q}")
                     for q in range(4)]
            tTx_q = [persist.tile([P, 8, 512], bf16, name=f"tTx{q}")
                     for q in range(4)]
            # persistent activations; strip t = douts [128t, 128t+128)
            # = head pair (2t, 2t+1)
            kTs = [persist.tile([P, NC], bf16, name=f"kT{t}")
                   for t in range(4)]
            qTs = [persist.tile([P, NQ], bf16, name=f"qT{t}")
                   for t in range(4)]
            # v strip per keytile, padded to 128 weight columns per head so
            # the AV matmul gets FWL: head h at cols [128h, 128h+64), ones
            # column at 128h+64, zeros elsewhere.
            v_exts = [persist.tile([P, HG * P], bf16, name=f"v_ext{kt}")
                      for kt in range(16)]

            # weights all resident
            wk_sb = w_pool.tile([P, 8, DG], bf16, tag="wk")
            wv_sb = w_pool.tile([P, 8, DG], bf16, tag="wv")
            wq_sb = w_pool.tile([P, 8, DG], bf16, tag="wq")

            def loop_a():
                if reps_a is None:
                    return contextlib.nullcontext()
                return tc.For_i(0, reps_a, 1)

            def loop_b():
                if reps_b is None:
                    return contextlib.nullcontext()
                return tc.For_i(0, reps_b, 1)

            def emit_kq_chunk(dst, w_sb, b_sb, tT_q, t, kc):
                # one [128, 512] chunk of kT/qT strip t (phase A path)
                pk = ps_acc.tile([P, 512], f32, tag="pacc",
                                 name=f"pk_{dst.name}_{kc}")
                for c in range(8):
                    nc.tensor.matmul(
                        pk[:],
                        w_sb[:, c, t * P:(t + 1) * P],
                        tT_q[kc][:, c, :],
                        start=(c == 0), stop=(c == 7))
                nc.vector.tensor_scalar_add(
                    dst[:, kc * 512:(kc + 1) * 512], pk[:], b_sb[:, t:t + 1])

            def emit_kq_granule(dst, w_sb, b_sb, tT_q, t, kc, g):
                # one [128, 128] sub-chunk (phase B filler granule)
                pk = ps_acc.tile([P, P], f32, tag="pacc",
                                 name=f"pg_{dst.name}_{kc}_{g}")
                for c in range(8):
                    nc.tensor.matmul(
                        pk[:],
                        w_sb[:, c, t * P:(t + 1) * P],
                        tT_q[kc][:, c, g * P:(g + 1) * P],
                        start=(c == 0), stop=(c == 7))
                col0 = kc * 512 + g * P
                nc.vector.tensor_scalar_add(
                    dst[:, col0:col0 + P], pk[:], b_sb[:, t:t + 1])

            def emit_v_strip(kt):
                # full 512-dout V strip for keytile kt (phase A path)
                pv = ps_acc.tile([P, 512], f32, tag="pacc", name=f"pv_{kt}")
                for c in range(8):
                    nc.tensor.matmul(
                        pv[:],
                        tTc_q[kt // 4][:, c, (kt % 4) * P:(kt % 4 + 1) * P],
                        wv_sb[:, c, :],
                        start=(c == 0), stop=(c == 7))
                nc.vector.tensor_add(
                    v_exts[kt][:].rearrange("p (h c) -> p h c", c=P)
                    [:, :, 0:64],
                    pv[:].rearrange("p (h c) -> p h c", c=64),
                    bvb_sb[:].rearrange("p (h c) -> p h c", c=64))

            def emit_v_granule(kt, g):
                # 128-dout (2-head) V granule (phase B filler)
                pv = ps_acc.tile([P, P], f32, tag="pacc",
                                 name=f"pvg_{kt}_{g}")
                for c in range(8):
                    nc.tensor.matmul(
                        pv[:],
                        tTc_q[kt // 4][:, c, (kt % 4) * P:(kt % 4 + 1) * P],
                        wv_sb[:, c, g * P:(g + 1) * P],
                        start=(c == 0), stop=(c == 7))
                nc.vector.tensor_add(
                    v_exts[kt][:].rearrange("p (h c) -> p h c", c=P)
                    [:, 2 * g:2 * g + 2, 0:64],
                    pv[:].rearrange("p (h c) -> p h c", c=64),
                    bvb_sb[:, g * P:(g + 1) * P]
                    .rearrange("p (h c) -> p h c", c=64))

            N_V_UPFRONT = 13

            # ---------------- Phase A ----------------
            with loop_a():
                # weights + biases ride the SWDGE (gpsimd) ring so the
                # HWDGE ring is free for the transpose loads
                nc.gpsimd.dma_start(bq_sb[:], bq2[:])
                nc.gpsimd.dma_start(bk_sb[:], bk2[:])
                nc.gpsimd.dma_start(bvb_sb[:], bvb[:])
                for c in range(8):
                    nc.gpsimd.dma_start(wk_sb[:, c, :], wk[c * P:(c + 1) * P, :])
                for c in range(8):
                    nc.gpsimd.dma_start(wv_sb[:, c, :], wv[c * P:(c + 1) * P, :])
                for c in range(8):
                    nc.gpsimd.dma_start(wq_sb[:, c, :], wq[c * P:(c + 1) * P, :])

                for q in range(4):
                    nc.sync.dma_start_transpose(
                        tTc_q[q][:], ctx_bf[512 * q:512 * (q + 1), :])
                for q in range(4):
                    nc.sync.dma_start_transpose(
                        tTx_q[q][:], x_bf[512 * q:512 * (q + 1), :])

                # zero the v_ext pads, set the ones columns (gpsimd+DVE)
                for kt in range(16):
                    nc.gpsimd.memset(v_exts[kt][:], 0.0)
                    nc.vector.tensor_copy(
                        v_exts[kt][:].rearrange("p (h c) -> p h c", c=P)
                        [:, :, 64],
                        ones_src[:])

                # kT strip 0 + V strips, interleaved to match quarter
                # arrival; then qT strip 0 (gated on the x quarters)
                for kc in range(4):
                    emit_kq_chunk(kTs[0], wk_sb, bk_sb, tTc_q, 0, kc)
                    for kt in range(4 * kc, min(4 * kc + 4, N_V_UPFRONT)):
                        emit_v_strip(kt)
                for qc in range(4):
                    emit_kq_chunk(qTs[0], wq_sb, bq_sb, tTx_q, 0, qc)

            # remaining work, injected as PE filler granules in phase B
            filler = []
            for kt in range(N_V_UPFRONT, 16):
                for g in range(4):
                    filler.append((emit_v_granule, (kt, g)))
            for t in (1, 2, 3):
                for kc in range(4):
                    for g in range(4):
                        filler.append((emit_kq_granule,
                                       (kTs[t], wk_sb, bk_sb, tTc_q, t, kc, g)))
                for kc in range(4):
                    for g in range(4):
                        filler.append((emit_kq_granule,
                                       (qTs[t], wq_sb, bq_sb, tTx_q, t, kc, g)))

            # ---------------- Phase B: attention ----------------
            with loop_b():
                fill_idx = [0]

                def maybe_fill():
                    if fill_idx[0] < len(filler):
                        fn, args = filler[fill_idx[0]]
                        fn(*args)
                        fill_idx[0] += 1

                pending_norm = [None]

                def flush_norm():
                    if pending_norm[0] is not None:
                        pending_norm[0]()
                        pending_norm[0] = None

                for hp in range(4):
                    for qc in range(4):
                        po = [ps_o.tile([P, 512], f32, tag=f"po{j}",
                                        name=f"po{hp}_{qc}_{j}")
                              for j in range(2)]

                        def emit_opair(at_prev, kt_prev, po=po, hp=hp):
                            for j in range(2):
                                nc.tensor.matmul(
                                    po[j][:],
                                    v_exts[kt_prev][
                                        :, (2 * hp + j) * P:
                                        (2 * hp + j + 1) * P],
                                    at_prev[:, j * 512:(j + 1) * 512],
                                    start=(kt_prev == 0),
                                    stop=(kt_prev == 15))

                        prev = None
                        for kt in range(16):
                            ps_pair = ps_acc.tile([P, 1024], f32, tag="pacc",
                                                  name=f"ps{hp}_{qc}_{kt}")
                            for j in range(2):
                                nc.tensor.matmul(
                                    ps_pair[:, j * 512:(j + 1) * 512],
                                    kTs[hp][j * 64:(j + 1) * 64,
                                            kt * P:(kt + 1) * P],
                                    qTs[hp][j * 64:(j + 1) * 64,
                                            qc * 512:(qc + 1) * 512],
                                    start=True, stop=True,
                                    tile_position=(j * 64, 0))
                            if kt == 1:
                                # normalize the previous q-chunk now; its
                                # PE op queues behind this chunk's scores
                                flush_norm()
                            else:
                                maybe_fill()
                            if prev is not None:
                                emit_opair(*prev)
                            at = att_pool.tile([P, 1024], bf16, tag="at",
                                               name=f"at{hp}_{qc}_{kt}")
                            nc.scalar.activation(at[:], ps_pair[:], AF.Exp)
                            prev = (at, kt)
                        emit_opair(*prev)

                        def norm(po=po, hp=hp, qc=qc):
                            for j in range(2):
                                rec = small_pool.tile(
                                    [1, 512], f32r, tag=f"rec{j}",
                                    name=f"rec{hp}_{qc}_{j}")
                                with nc.allow_low_precision(
                                        reason="f32r recip for bcast mm"):
                                    nc.vector.reciprocal(rec[:],
                                                         po[j][64:65, :])
                                pr = ps_acc.tile([64, 512], f32, tag="pacc",
                                                 name=f"pr{hp}_{qc}_{j}")
                                nc.tensor.matmul(pr[:], ones_col[:], rec[:],
                                                 start=True, stop=True)
                                rb = small_pool.tile([64, 512], f32,
                                                     tag=f"rb{j}",
                                                     name=f"rb{hp}_{qc}_{j}")
                                nc.vector.tensor_copy(rb[:], pr[:])
                                o_sb = out_pool.tile(
                                    [64, 512], f32, tag=f"o{j}",
                                    name=f"o_sb{hp}_{qc}_{j}")
                                nc.vector.tensor_mul(
                                    o_sb[:], po[j][0:64, :], rb[:])
                                h0 = (2 * hp + j) * 64
                                nc.sync.dma_start(
                                    out_T[h0:h0 + 64,
                                          qc * 512:(qc + 1) * 512],
                                    o_sb[:])
                        pending_norm[0] = norm
                    flush_norm()

    nc.compile()
    return nc


def _get_program():
    global _PROGRAM
    if _PROGRAM is None:
        _PROGRAM = _build_program()
    return _PROGRAM


def _numpy_fallback(x, context, mask, Wq, bq, Wk, bk, Wv, bv):
    out = np.empty((B, NQ, H * D), np.float32)
    for b in range(B):
        q = (x[b] @ Wq + bq).reshape(NQ, H, D)
        k = (context[b] @ Wk + bk).reshape(NC, H, D)
        v = (context[b] @ Wv + bv).reshape(NC, H, D)
        m = mask[b].astype(bool)
        for h in range(H):
            s = (q[:, h] @ k[:, h].T) * SCALE
            s = np.where(m[None, :], s, -np.finfo(np.float32).max)
            s = s - s.max(-1, keepdims=True)
            e = np.exp(s)
            a = e / e.sum(-1, keepdims=True)
            out[b, :, h * D:(h + 1) * D] = a @ v[:, h]
    return out


def make_in_maps(x, context, Wq, bq, Wk, bk, Wv, bv):
    import ml_dtypes
    BF = ml_dtypes.bfloat16
    x_bf = [np.ascontiguousarray(x[b].astype(BF)) for b in range(B)]
    c_bf = [np.ascontiguousarray(context[b].astype(BF)) for b in range(B)]
    in_maps = []
    for c in range(N_CORES):
        b, hg = divmod(c, 2)
        sl = slice(hg * DG, (hg + 1) * DG)
        in_maps.append({
            "x_bf": x_bf[b],
            "ctx_bf": c_bf[b],
            "wq": np.ascontiguousarray((Wq[:, sl] * SCALE).astype(BF)),
            "wk": np.ascontiguousarray(Wk[:, sl].astype(BF)),
            "wv": np.ascontiguousarray(Wv[:, sl].astype(BF)),
            # strip t of kT/qT gets bias for douts [128t, 128t+128)
            "bq2": np.ascontiguousarray(
                (bq[sl] * SCALE).reshape(4, P).T, np.float32),
            "bk2": np.ascontiguousarray(bk[sl].reshape(4, P).T, np.float32),
            "bvb": np.ascontiguousarray(
                np.broadcast_to(bv[sl], (P, DG)), np.float32),
        })
    return in_maps


def assemble_output(results):
    out = np.empty((B, NQ, H * D), np.float32)
    for c in range(N_CORES):
        b, hg = divmod(c, 2)
        out[b, :, hg * DG:(hg + 1) * DG] = results[c]["out_T"].T
    return out


def kernel(x, context, mask, Wq, bq, Wk, bk, Wv, bv):
    x = np.asarray(x, np.float32)
    context = np.asarray(context, np.float32)
    mask = np.asarray(mask)
    Wq = np.asarray(Wq, np.float32)
    bq = np.asarray(bq, np.float32)
    Wk = np.asarray(Wk, np.float32)
    bk = np.asarray(bk, np.float32)
    Wv = np.asarray(Wv, np.float32)
    bv = np.asarray(bv, np.float32)

    if not mask.all():
        return _numpy_fallback(x, context, mask, Wq, bq, Wk, bk, Wv, bv)

    from concourse.bass_utils import run_bass_kernel_spmd

    nc = _get_program()
    in_maps = make_in_maps(x, context, Wq, bq, Wk, bk, Wv, bv)
    res = run_bass_kernel_spmd(nc, in_maps, core_ids=list(range(N_CORES)))
    return assemble_output(res.results)


# revision 14
# speedup vs baseline: 1.1338x; 1.0739x over previous
"""CrossAttention Trainium2 kernel.

Full inputs -> full output. Sharding: 8 cores = 4 batches x 2 head-groups
(8 heads each). Per core:

  Inputs x/context arrive pre-cast to bf16 (host) and are loaded with the
  X-bar DMA-transpose directly into cin-major layout (no PE transposes).
  Weights arrive pre-cast bf16 (wq pre-scaled by D**-0.5).

  Phase A: DMA-transpose x/ctx (4 quarter-tensor calls each, HWDGE ring),
  weights on the SWDGE ring, then PE projects kT strip 0, V strips 0-12
  and qT strip 0, each chunk gated only on its input quarter.

  Phase B (ACT-paced): per head-pair/q-chunk/key-tile
    scoresT[key, qrow] = kT.T @ qT   (two heads concurrent on disjoint PE
                                      row groups via tile_position)
    attnT = exp(scoresT)             (no max-subtraction: |scores| <~ 3)
    O^T += v_ext.T @ attnT           v_ext padded to 128 weight columns
                                     (FWL) with a ones column at col 64
                                     -> po row 64 = softmax denominator
    out = po[0:64] * (1/po[64])      broadcast via K=1 matmul

  Remaining projections (V strips 13-15, kT/qT strips 1-3) are split into
  ~450ns granules (8 matmuls of n=128) and injected one per iteration as
  PE filler inside the ACT-paced loop.
"""

import numpy as np

B, NQ, NC = 4, 2048, 2048
QDIM = CDIM = 1024
H, D = 16, 64
SCALE = D**-0.5
P = 128
HG = 8            # heads per core
DG = HG * D       # 512 output dims per core
N_CORES = 8

_PROGRAM = None


def _build_program(reps_a=None, reps_b=None):
    import contextlib
    import concourse.mybir as mybir
    import concourse.tile as tile
    from concourse import bacc

    f32 = mybir.dt.float32
    f32r = mybir.dt.float32r
    bf16 = mybir.dt.bfloat16
    AF = mybir.ActivationFunctionType

    nc = bacc.Bacc("TRN2", target_bir_lowering=False, debug=False,
                   num_devices=N_CORES)

    x_bf = nc.dram_tensor("x_bf", [NQ, QDIM], bf16, kind="ExternalInput")
    ctx_bf = nc.dram_tensor("ctx_bf", [NC, CDIM], bf16, kind="ExternalInput")
    wq = nc.dram_tensor("wq", [QDIM, DG], bf16, kind="ExternalInput")
    wk = nc.dram_tensor("wk", [CDIM, DG], bf16, kind="ExternalInput")
    wv = nc.dram_tensor("wv", [CDIM, DG], bf16, kind="ExternalInput")
    bq2 = nc.dram_tensor("bq2", [P, 4], f32, kind="ExternalInput")
    bk2 = nc.dram_tensor("bk2", [P, 4], f32, kind="ExternalInput")
    bvb = nc.dram_tensor("bvb", [P, DG], f32, kind="ExternalInput")
    out_T = nc.dram_tensor("out_T", [DG, NQ], f32, kind="ExternalOutput")

    with tile.TileContext(nc) as tc:
        with (
            tc.tile_pool(name="const", bufs=1) as const_pool,
            tc.tile_pool(name="persist", bufs=1) as persist,
            tc.tile_pool(name="wpool", bufs=1) as w_pool,
            tc.tile_pool(name="att", bufs=4) as att_pool,
            tc.tile_pool(name="outp", bufs=2) as out_pool,
            tc.tile_pool(name="small", bufs=2) as small_pool,
            tc.tile_pool(name="ps_acc", bufs=2, space="PSUM") as ps_acc,
            tc.tile_pool(name="ps_fill", bufs=1, space="PSUM") as ps_fill,
            tc.tile_pool(name="ps_o", bufs=1, space="PSUM") as ps_o,
        ):
            ones_f32 = const_pool.tile([1, 64], f32)
            nc.vector.memset(ones_f32[:], 1.0)
            ones_col = const_pool.tile([1, 64], f32r)
            nc.vector.tensor_copy(ones_col[:], ones_f32[:])
            bq_sb = const_pool.tile([P, 4], f32)
            bk_sb = const_pool.tile([P, 4], f32)
            bvb_sb = const_pool.tile([P, DG], f32)
            ones_src = const_pool.tile([P, HG], f32)
            nc.vector.memset(ones_src[:], 1.0)

            # transposed inputs, one tile per 512-row quarter:
            # tT*_q[q][cin%128, cin//128, row-512q] = src[row, cin]
            tTc_q = [persist.tile([P, 8, 512], bf16, name=f"tTc{q}")
                     for q in range(4)]
            tTx_q = [persist.tile([P, 8, 512], bf16, name=f"tTx{q}")
                     for q in range(4)]
            # persistent activations; strip t = douts [128t, 128t+128)
            # = head pair (2t, 2t+1)
            kTs = [persist.tile([P, NC], bf16, name=f"kT{t}")
                   for t in range(4)]
            qTs = [persist.tile([P, NQ], bf16, name=f"qT{t}")
                   for t in range(4)]
            # v strip per keytile, padded to 128 weight columns per head so
            # the AV matmul gets FWL: head h at cols [128h, 128h+64), ones
            # column at 128h+64, zeros elsewhere.
            v_exts = [persist.tile([P, HG * P], bf16, name=f"v_ext{kt}")
                      for kt in range(16)]

            # weights all resident
            wk_sb = w_pool.tile([P, 8, DG], bf16, tag="wk")
            wv_sb = w_pool.tile([P, 8, DG], bf16, tag="wv")
            wq_sb = w_pool.tile([P, 8, DG], bf16, tag="wq")

            def loop_a():
                if reps_a is None:
                    return contextlib.nullcontext()
                return tc.For_i(0, reps_a, 1)

            def loop_b():
                if reps_b is None:
                    return contextlib.nullcontext()
                return tc.For_i(0, reps_b, 1)

            def emit_kq_chunk(dst, w_sb, b_sb, tT_q, t, kc):
                # one [128, 512] chunk of kT/qT strip t (phase A path)
                pk = ps_acc.tile([P, 512], f32, tag="pacc",
                                 name=f"pk_{dst.name}_{kc}")
                for c in range(8):
                    nc.tensor.matmul(
                        pk[:],
                        w_sb[:, c, t * P:(t + 1) * P],
                        tT_q[kc][:, c, :],
                        start=(c == 0), stop=(c == 7))
                nc.vector.tensor_scalar_add(
                    dst[:, kc * 512:(kc + 1) * 512], pk[:], b_sb[:, t:t + 1])

            fill_pk = [None]

            def emit_kq_granule(dst, w_sb, b_sb, tT_q, t, kc, pair):
                # 2 of the 8 cin accumulation matmuls of a [128, 512] chunk
                # (phase B filler granule; the group stays open between
                # granules, like the po accumulation does)
                if pair == 0:
                    fill_pk[0] = ps_fill.tile([P, 512], f32, tag="pk",
                                              name=f"pg_{dst.name}_{kc}")
                pk = fill_pk[0]
                for c in (2 * pair, 2 * pair + 1):
                    nc.tensor.matmul(
                        pk[:],
                        w_sb[:, c, t * P:(t + 1) * P],
                        tT_q[kc][:, c, :],
                        start=(c == 0), stop=(c == 7))
                if pair == 3:
                    nc.vector.tensor_scalar_add(
                        dst[:, kc * 512:(kc + 1) * 512], pk[:],
                        b_sb[:, t:t + 1])

            def emit_v_strip(kt):
                # full 512-dout V strip for keytile kt (phase A path)
                pv = ps_acc.tile([P, 512], f32, tag="pacc", name=f"pv_{kt}")
                for c in range(8):
                    nc.tensor.matmul(
                        pv[:],
                        tTc_q[kt // 4][:, c, (kt % 4) * P:(kt % 4 + 1) * P],
                        wv_sb[:, c, :],
                        start=(c == 0), stop=(c == 7))
                nc.vector.tensor_add(
                    v_exts[kt][:].rearrange("p (h c) -> p h c", c=P)
                    [:, :, 0:64],
                    pv[:].rearrange("p (h c) -> p h c", c=64),
                    bvb_sb[:].rearrange("p (h c) -> p h c", c=64))

            def emit_v_granule(kt, pair):
                # 2 of the 8 cin accumulation matmuls of V strip kt
                if pair == 0:
                    fill_pk[0] = ps_fill.tile([P, 512], f32, tag="pk",
                                              name=f"pvg_{kt}")
                pv = fill_pk[0]
                for c in (2 * pair, 2 * pair + 1):
                    nc.tensor.matmul(
                        pv[:],
                        tTc_q[kt // 4][:, c, (kt % 4) * P:(kt % 4 + 1) * P],
                        wv_sb[:, c, :],
                        start=(c == 0), stop=(c == 7))
                if pair == 3:
                    nc.vector.tensor_add(
                        v_exts[kt][:].rearrange("p (h c) -> p h c", c=P)
                        [:, :, 0:64],
                        pv[:].rearrange("p (h c) -> p h c", c=64),
                        bvb_sb[:].rearrange("p (h c) -> p h c", c=64))

            N_V_UPFRONT = 13

            # ---------------- Phase A ----------------
            with loop_a():
                # DMA emission order == service order in practice, so
                # interleave: first ctx quarter, then weights (coalesced
                # HWDGE, one per tensor, scalar ring), then alternating
                # ctx/x quarters so kT0/V work streams right behind DMA
                nc.scalar.dma_start(bq_sb[:], bq2[:])
                nc.scalar.dma_start(bk_sb[:], bk2[:])
                nc.scalar.dma_start(bvb_sb[:], bvb[:])
                nc.sync.dma_start_transpose(
                    tTc_q[0][:], ctx_bf[0:512, :])
                nc.scalar.dma_start(
                    wk_sb[:], wk[:].rearrange("(c p) d -> p c d", p=P))
                nc.scalar.dma_start(
                    wv_sb[:], wv[:].rearrange("(c p) d -> p c d", p=P))
                nc.scalar.dma_start(
                    wq_sb[:], wq[:].rearrange("(c p) d -> p c d", p=P))
                for q in (1, 2, 3):
                    nc.sync.dma_start_transpose(
                        tTc_q[q][:], ctx_bf[512 * q:512 * (q + 1), :])
                    nc.sync.dma_start_transpose(
                        tTx_q[q - 1][:], x_bf[512 * (q - 1):512 * q, :])
                nc.sync.dma_start_transpose(
                    tTx_q[3][:], x_bf[1536:2048, :])

                # zero the v_ext pads, set the ones columns (gpsimd+DVE)
                for kt in range(16):
                    nc.gpsimd.memset(v_exts[kt][:], 0.0)
                    nc.vector.tensor_copy(
                        v_exts[kt][:].rearrange("p (h c) -> p h c", c=P)
                        [:, :, 64],
                        ones_src[:])

                # kT strip 0 + V strips, interleaved to match quarter
                # arrival; then qT strip 0 (gated on the x quarters)
                for kc in range(4):
                    emit_kq_chunk(kTs[0], wk_sb, bk_sb, tTc_q, 0, kc)
                    for kt in range(4 * kc, min(4 * kc + 4, N_V_UPFRONT)):
                        emit_v_strip(kt)
                for qc in range(4):
                    emit_kq_chunk(qTs[0], wq_sb, bq_sb, tTx_q, 0, qc)

            # remaining work, injected as PE filler granules in phase B
            filler = []
            for kt in range(N_V_UPFRONT, 16):
                for pair in range(4):
                    filler.append((emit_v_granule, (kt, pair)))
            for t in (1, 2, 3):
                for kc in range(4):
                    for pair in range(4):
                        filler.append((emit_kq_granule,
                                       (kTs[t], wk_sb, bk_sb, tTc_q, t, kc,
                                        pair)))
                for kc in range(4):
                    for pair in range(4):
                        filler.append((emit_kq_granule,
                                       (qTs[t], wq_sb, bq_sb, tTx_q, t, kc,
                                        pair)))

            # ---------------- Phase B: attention ----------------
            with loop_b():
                fill_idx = [0]
                it = [0]

                def quota():
                    # granule pacing: V strips 1/iter up front (tight
                    # deadlines), then ~0.65/iter, which keeps per-iter PE
                    # work near the ACT pace instead of front-loading a
                    # PE-bound prefix
                    i = it[0]
                    q = min(i, 12)
                    if i > 12:
                        q += int((i - 12) * 0.65)
                    return min(q, len(filler))

                def maybe_fill(budget=2):
                    for _ in range(budget):
                        if fill_idx[0] < quota():
                            fn, args = filler[fill_idx[0]]
                            fn(*args)
                            fill_idx[0] += 1

                pending_norm = [None]

                def flush_norm():
                    if pending_norm[0] is not None:
                        pending_norm[0]()
                        pending_norm[0] = None

                for hp in range(4):
                    for qc in range(4):
                        po = [ps_o.tile([P, 512], f32, tag=f"po{j}",
                                        name=f"po{hp}_{qc}_{j}")
                              for j in range(2)]

                        def emit_opair(at_prev, kt_prev, po=po, hp=hp):
                            for j in range(2):
                                nc.tensor.matmul(
                                    po[j][:],
                                    v_exts[kt_prev][
                                        :, (2 * hp + j) * P:
                                        (2 * hp + j + 1) * P],
                                    at_prev[:, j * 512:(j + 1) * 512],
                                    start=(kt_prev == 0),
                                    stop=(kt_prev == 15))

                        prev = None
                        for kt in range(16):
                            ps_pair = ps_acc.tile([P, 1024], f32, tag="pacc",
                                                  name=f"ps{hp}_{qc}_{kt}")
                            for j in range(2):
                                nc.tensor.matmul(
                                    ps_pair[:, j * 512:(j + 1) * 512],
                                    kTs[hp][j * 64:(j + 1) * 64,
                                            kt * P:(kt + 1) * P],
                                    qTs[hp][j * 64:(j + 1) * 64,
                                            qc * 512:(qc + 1) * 512],
                                    start=True, stop=True,
                                    tile_position=(j * 64, 0))
                            if kt == 1:
                                # normalize the previous q-chunk now; its
                                # PE op queues behind this chunk's scores
                                flush_norm()
                            else:
                                maybe_fill()
                            it[0] += 1
                            if prev is not None:
                                emit_opair(*prev)
                            at = att_pool.tile([P, 1024], bf16, tag="at",
                                               name=f"at{hp}_{qc}_{kt}")
                            nc.scalar.activation(at[:], ps_pair[:], AF.Exp)
                            prev = (at, kt)
                        emit_opair(*prev)
                        # stage po out of PSUM right away so the next
                        # q-chunk's accumulation can reuse the po slot
                        # without waiting for the deferred normalization
                        stage = [small_pool.tile([65, 512], f32,
                                                 tag=f"st{j}",
                                                 name=f"st{hp}_{qc}_{j}")
                                 for j in range(2)]
                        for j in range(2):
                            nc.vector.tensor_copy(stage[j][:],
                                                  po[j][0:65, :])

                        def norm(stage=stage, hp=hp, qc=qc):
                            for j in range(2):
                                rec = small_pool.tile(
                                    [1, 512], f32r, tag=f"rec{j}",
                                    name=f"rec{hp}_{qc}_{j}")
                                with nc.allow_low_precision(
                                        reason="f32r recip for bcast mm"):
                                    nc.vector.reciprocal(rec[:],
                                                         stage[j][64:65, :])
                                pr = ps_fill.tile([64, 512], f32, tag="pr",
                                                  name=f"pr{hp}_{qc}_{j}")
                                nc.tensor.matmul(pr[:], ones_col[:], rec[:],
                                                 start=True, stop=True)
                                rb = small_pool.tile([64, 512], f32,
                                                     tag=f"rb{j}",
                                                     name=f"rb{hp}_{qc}_{j}")
                                nc.vector.tensor_copy(rb[:], pr[:])
                                o_sb = out_pool.tile(
                                    [64, 512], f32, tag=f"o{j}",
                                    name=f"o_sb{hp}_{qc}_{j}")
                                nc.vector.tensor_mul(
                                    o_sb[:], stage[j][0:64, :], rb[:])
                                h0 = (2 * hp + j) * 64
                                nc.sync.dma_start(
                                    out_T[h0:h0 + 64,
                                          qc * 512:(qc + 1) * 512],
                                    o_sb[:])
                        pending_norm[0] = norm
                    flush_norm()

    nc.compile()
    return nc


def _get_program():
    global _PROGRAM
    if _PROGRAM is None:
        _PROGRAM = _build_program()
    return _PROGRAM


def _numpy_fallback(x, context, mask, Wq, bq, Wk, bk, Wv, bv):
    out = np.empty((B, NQ, H * D), np.float32)
    for b in range(B):
        q = (x[b] @ Wq + bq).reshape(NQ, H, D)
        k = (context[b] @ Wk + bk).reshape(NC, H, D)
        v = (context[b] @ Wv + bv).reshape(NC, H, D)
        m = mask[b].astype(bool)
        for h in range(H):
            s = (q[:, h] @ k[:, h].T) * SCALE
            s = np.where(m[None, :], s, -np.finfo(np.float32).max)
            s = s - s.max(-1, keepdims=True)
            e = np.exp(s)
            a = e / e.sum(-1, keepdims=True)
            out[b, :, h * D:(h + 1) * D] = a @ v[:, h]
    return out


def make_in_maps(x, context, Wq, bq, Wk, bk, Wv, bv):
    import ml_dtypes
    BF = ml_dtypes.bfloat16
    x_bf = [np.ascontiguousarray(x[b].astype(BF)) for b in range(B)]
    c_bf = [np.ascontiguousarray(context[b].astype(BF)) for b in range(B)]
    in_maps = []
    for c in range(N_CORES):
        b, hg = divmod(c, 2)
        sl = slice(hg * DG, (hg + 1) * DG)
        in_maps.append({
            "x_bf": x_bf[b],
            "ctx_bf": c_bf[b],
            "wq": np.ascontiguousarray((Wq[:, sl] * SCALE).astype(BF)),
            "wk": np.ascontiguousarray(Wk[:, sl].astype(BF)),
            "wv": np.ascontiguousarray(Wv[:, sl].astype(BF)),
            # strip t of kT/qT gets bias for douts [128t, 128t+128)
            "bq2": np.ascontiguousarray(
                (bq[sl] * SCALE).reshape(4, P).T, np.float32),
            "bk2": np.ascontiguousarray(bk[sl].reshape(4, P).T, np.float32),
            "bvb": np.ascontiguousarray(
                np.broadcast_to(bv[sl], (P, DG)), np.float32),
        })
    return in_maps


def assemble_output(results):
    out = np.empty((B, NQ, H * D), np.float32)
    for c in range(N_CORES):
        b, hg = divmod(c, 2)
        out[b, :, hg * DG:(hg + 1) * DG] = results[c]["out_T"].T
    return out


def kernel(x, context, mask, Wq, bq, Wk, bk, Wv, bv):
    x = np.asarray(x, np.float32)
    context = np.asarray(context, np.float32)
    mask = np.asarray(mask)
    Wq = np.asarray(Wq, np.float32)
    bq = np.asarray(bq, np.float32)
    Wk = np.asarray(Wk, np.float32)
    bk = np.asarray(bk, np.float32)
    Wv = np.asarray(Wv, np.float32)
    bv = np.asarray(bv, np.float32)

    if not mask.all():
        return _numpy_fallback(x, context, mask, Wq, bq, Wk, bk, Wv, bv)

    from concourse.bass_utils import run_bass_kernel_spmd

    nc = _get_program()
    in_maps = make_in_maps(x, context, Wq, bq, Wk, bk, Wv, bv)
    res = run_bass_kernel_spmd(nc, in_maps, core_ids=list(range(N_CORES)))
    return assemble_output(res.results)
